# revision 1
# baseline (speedup 1.0000x reference)
"""Bass/Trainium2 kernel for nn_Graph_Layer (gnn_message_passing).

Reference math (N=8192, D=512):
    G0[i,j] = ||s_i - s_j + eps||_2   (pairwise distances, Gram trick)
    G = 1 - G0 / rowmax(G0)
    out = (G @ x) @ W

Decomposition used here (row-shard over 8 cores, 1024 rows each):
    sqd[i,j] = ri[i] + cj[j] - 2*gram[i,j]        (ri, cj host-precomputed)
    G0 = sqrt(sqd + CLAMP)                         (CLAMP covers tf32 noise on diag)
    rowmax[i] = max_j G0[i,j]
    (G @ x)[i,:] = colsum_x - Y0[i,:]/rowmax[i],   Y0 = G0 @ x
    out[i,:]  = w2 - (Y0[i,:]/rowmax[i]) @ W,      w2 = colsum_x @ W (host)

On device, the distance strip is computed TRANSPOSED (sqd^T[j,i]) so that the
G0 tiles come out with j (the contraction dim of Y0 = G0 @ x) on partitions --
no transposes of G0 needed. Each core sees its own np.roll'ed copy of the
inputs so the "local rows" are always rows [0,1024): a single uniform SPMD
program runs on all 8 cores.

All matmuls use float32r (TF32 mode: 1 cycle/row at free dim >= 512).
"""

import numpy as np
from contextlib import ExitStack

import concourse.bass as bass
from concourse import bacc
import concourse.tile as tile
from concourse import mybir
from concourse.bass_utils import run_bass_kernel_spmd
from concourse.masks import make_identity

N, D, NOUT = 8192, 512, 512
M = 8                 # cores
R = N // M            # 1024 local rows per core
EPS = 1e-6
CLAMP = 0.3           # covers tf32 rounding noise on the diagonal; ~1e-4 rel effect off-diag
F32 = mybir.dt.float32
F32R = mybir.dt.float32r

KT = D // 128         # 4 contraction sub-tiles
NJT = N // 128        # 64 j tiles
IB = 512              # i block (free dim of the gram matmuls)
NIB = R // IB         # 2
NSUB = IB // 128      # 4 sub-tiles of 128 rows per i block

CH = 512              # S^T DMA chunk width (columns); chunk c covers j_tiles 4c..4c+3
NCH = N // CH
LOOKAHEAD = 4         # chunks issued ahead of consumption


def build_kernel(ctx, tc, out_d, x_d, s_d, cj_d, ri_d, w_d):
    nc = tc.nc

    singles = ctx.enter_context(tc.tile_pool(name="singles", bufs=1))
    xt_pool = ctx.enter_context(tc.tile_pool(name="xt", bufs=4))
    g0_pool = ctx.enter_context(tc.tile_pool(name="g0", bufs=3))
    ysc_pool = ctx.enter_context(tc.tile_pool(name="ysc", bufs=4))
    yscT_pool = ctx.enter_context(tc.tile_pool(name="ysct", bufs=2))
    osb_pool = ctx.enter_context(tc.tile_pool(name="osb", bufs=2))
    sm_pool = ctx.enter_context(tc.tile_pool(name="sm", bufs=4))
    macc_pool = ctx.enter_context(tc.tile_pool(name="macc", bufs=2))
    ps_tr = ctx.enter_context(tc.tile_pool(name="ps_tr", bufs=2, space="PSUM"))
    ps_g = ctx.enter_context(tc.tile_pool(name="ps_g", bufs=2, space="PSUM"))
    ps_y = ctx.enter_context(tc.tile_pool(name="ps_y", bufs=1, space="PSUM"))

    # --- persistent SBUF tensors ---
    st = singles.tile([128, KT * N], F32R)            # S^T: [k*N + j] layout
    w_sb = singles.tile([128, 5 * NOUT], F32R)        # W rows 0..511 + w2 row (tile 4, part 0)
    cj_sb = singles.tile([128, NJT], F32)             # cj[t*128+p] at [p, t]
    ri_sb = singles.tile([1, R], F32R)                # -ri/2
    ones_sb = singles.tile([1, 128], F32R)
    ident = singles.tile([128, 128], F32)

    ones32 = singles.tile([1, 128], F32)
    nc.vector.memset(ones32[:], 1.0)
    nc.vector.tensor_copy(out=ones_sb[:], in_=ones32[:])
    make_identity(nc, ident[:])
    def load_st_chunk(c):
        for k in range(KT):
            nc.sync.dma_start(
                out=st[:, k * N + c * CH: k * N + (c + 1) * CH],
                in_=s_d[bass.ts(k, 128), c * CH:(c + 1) * CH].bitcast(F32R),
            )

    nc.sync.dma_start(out=ri_sb[:], in_=ri_d.bitcast(F32R))
    load_st_chunk(0)
    nc.sync.dma_start(out=cj_sb[:], in_=cj_d)

    # --- main: per i-block: gram strip -> G0 -> Y0 accum -> normalize -> GEMM ---
    for ib in range(NIB):
        icol0 = ib * IB  # local column offset into S^T / ri
        psy = [ps_y.tile([128, NOUT], F32, tag=f"y{s}", name=f"psy{s}")
               for s in range(NSUB)]
        macc = macc_pool.tile([128, IB], F32, tag="macc")

        for jt in range(NJT):
            xt = xt_pool.tile([128, D], F32R, tag="xt")
            nc.sync.dma_start(out=xt[:], in_=x_d[bass.ts(jt, 128), :].bitcast(F32R))

            if ib == 0:
                if jt == 0:
                    load_st_chunk(1)
                    load_st_chunk(2)
                elif jt % 4 == 0 and jt // 4 + 2 < NCH:
                    load_st_chunk(jt // 4 + 2)
                if jt == 32:
                    for kt in range(5):
                        nc.sync.dma_start(
                            out=w_sb[:, kt * NOUT:(kt + 1) * NOUT],
                            in_=w_d[bass.ts(kt, 128), :].bitcast(F32R),
                        )

            psg = ps_g.tile([128, IB], F32, tag="g")
            for k in range(KT):
                nc.tensor.matmul(
                    psg[:],
                    st[:, k * N + jt * 128: k * N + jt * 128 + 128],
                    st[:, k * N + icol0: k * N + icol0 + IB],
                    start=(k == 0),
                    stop=False,
                )
            # aug row: += 1 * (-ri[i]/2)
            nc.tensor.matmul(
                psg[:], ones_sb[:], ri_sb[:, icol0:icol0 + IB],
                start=False, stop=True,
            )

            # G0^T tile = sqrt(-2*psg + cj[j])   (cj includes +CLAMP)
            g0 = g0_pool.tile([128, IB], F32R, tag="g0")
            nc.scalar.activation(
                out=g0[:], in_=psg[:],
                func=mybir.ActivationFunctionType.Sqrt,
                bias=cj_sb[:, jt:jt + 1], scale=-2.0,
            )

            if jt == 0:
                nc.vector.tensor_copy(out=macc[:], in_=g0[:].bitcast(F32))
            else:
                nc.vector.tensor_max(macc[:], macc[:], g0[:].bitcast(F32))

            # software pipeline: issue Y matmuls one step behind the gram so
            # the PE fills the ACT sqrt latency with the next gram
            if jt > 0:
                pg0, pxt = prev
                for s in range(NSUB):
                    nc.tensor.matmul(
                        psy[s][:], pg0[:, bass.ts(s, 128)], pxt[:],
                        start=(jt == 1), stop=False,
                    )
            prev = (g0, xt)

        pg0, pxt = prev
        for s in range(NSUB):
            nc.tensor.matmul(
                psy[s][:], pg0[:, bass.ts(s, 128)], pxt[:],
                start=False, stop=True,
            )

        # tail, part 1: rowmax -> -1/rowmax -> scale Y out of PSUM (frees psy fast)
        yscs = []
        for s in range(NSUB):
            pst = ps_tr.tile([128, 128], F32, tag="tr")
            nc.tensor.transpose(pst[:], macc[:, bass.ts(s, 128)], ident[:])
            rm = sm_pool.tile([128, 1], F32, tag="rm")
            nc.vector.tensor_reduce(
                out=rm[:], in_=pst[:], axis=mybir.AxisListType.X,
                op=mybir.AluOpType.max,
            )
            nrm = sm_pool.tile([128, 1], F32, tag="nrm")
            nc.vector.tensor_scalar_mul(nrm[:], rm[:], -1.0)
            ninv = sm_pool.tile([128, 1], F32, tag="ninv")
            nc.vector.reciprocal(ninv[:], nrm[:])  # -1/rowmax

            ysc = ysc_pool.tile([128, NOUT], F32, tag="ysc", name=f"ysc{s}")
            nc.scalar.activation(
                out=ysc[:], in_=psy[s][:],
                func=mybir.ActivationFunctionType.Copy, scale=ninv[:],
            )
            yscs.append(ysc)

        # tail, part 2: transpose Ysc and multiply by W (+ w2 aug row)
        for s in range(NSUB):
            ysc = yscs[s]
            ysct = yscT_pool.tile([128, KT * 128], F32R, tag="ysct")
            for k in range(KT):
                pst2 = ps_tr.tile([128, 128], F32, tag="tr")
                nc.tensor.transpose(pst2[:], ysc[:, bass.ts(k, 128)], ident[:])
                if k % 2 == 0:
                    nc.vector.tensor_copy(out=ysct[:, bass.ts(k, 128)], in_=pst2[:])
                else:
                    nc.scalar.copy(out=ysct[:, bass.ts(k, 128)], in_=pst2[:])

            pso = ps_g.tile([128, NOUT], F32, tag="g", name=f"pso{s}")
            for k in range(KT):
                nc.tensor.matmul(
                    pso[:],
                    ysct[:, bass.ts(k, 128)],
                    w_sb[:, k * NOUT:(k + 1) * NOUT],
                    start=(k == 0),
                    stop=False,
                )
            nc.tensor.matmul(
                pso[:], ones_sb[:], w_sb[0:1, 4 * NOUT:5 * NOUT],
                start=False, stop=True,
            )
            osb = osb_pool.tile([128, NOUT], F32, tag="osb")
            nc.vector.tensor_copy(out=osb[:], in_=pso[:])
            nc.sync.dma_start(out=out_d[bass.ts(ib * NSUB + s, 128), :], in_=osb[:])


_NC_CACHE = {}


def _build_nc():
    if "nc" in _NC_CACHE:
        return _NC_CACHE["nc"]
    nc = bacc.Bacc("TRN2", target_bir_lowering=False, debug=False, num_devices=M)
    x_d = nc.dram_tensor("x", [N, D], F32, kind="ExternalInput").ap()
    s_d = nc.dram_tensor("simT", [D, N], F32, kind="ExternalInput").ap()
    cj_d = nc.dram_tensor("cj", [128, NJT], F32, kind="ExternalInput").ap()
    ri_d = nc.dram_tensor("riaug", [1, R], F32, kind="ExternalInput").ap()
    w_d = nc.dram_tensor("waug", [640, NOUT], F32, kind="ExternalInput").ap()
    out_d = nc.dram_tensor("out", [R, NOUT], F32, kind="ExternalOutput").ap()
    with tile.TileContext(nc) as tc, ExitStack() as ctx:
        build_kernel(ctx, tc, out_d, x_d, s_d, cj_d, ri_d, w_d)
    nc.compile()
    _NC_CACHE["nc"] = nc
    return nc


def make_in_maps(x, sim_feat, weight):
    x = np.ascontiguousarray(x, dtype=np.float32)
    sim = np.ascontiguousarray(sim_feat, dtype=np.float32)
    w = np.ascontiguousarray(weight, dtype=np.float32)

    sim64 = sim.astype(np.float64)
    sq = (sim64 * sim64).sum(1)
    ss = sim64.sum(1)
    cj_full = (sq - 2.0 * EPS * ss + CLAMP).astype(np.float32)         # [N]
    ri_full = sq + 2.0 * EPS * ss + D * EPS * EPS                      # [N] f64
    colsum = x.astype(np.float64).sum(0)
    w2 = (colsum @ w.astype(np.float64)).astype(np.float32)
    waug = np.zeros((640, NOUT), np.float32)
    waug[:D] = w
    waug[D] = w2

    in_maps = []
    for c in range(M):
        shift = c * R
        sim_c = np.ascontiguousarray(np.roll(sim, -shift, axis=0).T)
        x_c = np.roll(x, -shift, axis=0)
        cj_c = np.ascontiguousarray(
            np.roll(cj_full, -shift).reshape(NJT, 128).T
        )                                                               # [128, NJT]
        ri_c = np.ascontiguousarray(
            (-(ri_full[shift:shift + R]) / 2.0).astype(np.float32).reshape(1, R)
        )
        in_maps.append(
            {"x": x_c, "simT": sim_c, "cj": cj_c, "riaug": ri_c, "waug": waug}
        )
    return in_maps


def kernel(x, sim_feat, weight, _trace=False, **kw):
    nc = _build_nc()
    in_maps = make_in_maps(x, sim_feat, weight)
    res = run_bass_kernel_spmd(nc, in_maps, list(range(M)), trace=_trace, **kw)
    out = np.concatenate([res.results[c]["out"] for c in range(M)], axis=0)
    if _trace:
        return out, res
    return out



# revision 7
# speedup vs baseline: 1.1496x; 1.1496x over previous
"""Bass/Trainium2 kernel for nn_Graph_Layer (gnn_message_passing).

Reference math (N=8192, D=512):
    G0[i,j] = ||s_i - s_j + eps||_2   (pairwise distances, Gram trick)
    G = 1 - G0 / rowmax(G0)
    out = (G @ x) @ W

Decomposition (row-shard over 8 cores, 1024 rows each):
    sqd[i,j] = ri[i] + cj[j] - 2*gram[i,j]        (ri, cj host-precomputed)
    G0 = sqrt(sqd + CLAMP)                         (CLAMP covers tf32 noise on diag)
    rowmax[i] = max_j G0[i,j]
    (G @ x)[i,:] = colsum_x - Y0[i,:]/rowmax[i],   Y0 = G0 @ x
    out[i,:]  = w2 - (Y0 @ W)[i,:]/rowmax[i],      w2 = colsum_x @ W (host)

On device the distance strip is computed TRANSPOSED (sqd^T[j,i]) so G0 tiles
come out with j (the contraction dim of Y0) on partitions. The per-i "ri" term
is added by the Vector engine from a host-precomputed broadcast tile (riB)
instead of a 1-row PE matmul: a 1-row matmul costs the same PE cycles as a
full one (cost = output free size), and its LDWEIGHTS stalled the PE pipeline
every iteration, dropping the p-state.

Y0 is accumulated transposed (Y0T[c,i], stationary = x c-slices, moving = G0)
so no transposes are needed before the W GEMM; the GEMM output lands [i, n]
with i on partitions, where the -1/rowmax scale is a per-partition ACT scale
and w2 is a DVE add of a host broadcast tile.

Each core sees its own np.roll'ed copy of the inputs so local rows are always
[0,1024): a single uniform SPMD program runs on all 8 cores. All matmuls use
float32r (TF32 mode).
"""

import numpy as np
from contextlib import ExitStack

import concourse.bass as bass
from concourse import bacc
import concourse.tile as tile
from concourse import mybir
from concourse.bass_utils import run_bass_kernel_spmd
from concourse.masks import make_identity

N, D, NOUT = 8192, 512, 512
M = 8                 # cores
R = N // M            # 1024 local rows per core
EPS = 1e-6
CLAMP = 0.3           # covers tf32 rounding noise on the diagonal; ~1e-4 rel effect off-diag
F32 = mybir.dt.float32
F32R = mybir.dt.float32r

KT = D // 128         # 4 contraction sub-tiles
NJT = N // 128        # 64 j tiles
IB = 512              # i block (free dim of the gram matmuls)
NIB = R // IB         # 2
NSUB = IB // 128      # 4 sub-tiles of 128 rows per i block

CH = 512              # S^T DMA chunk width (columns); chunk c covers j_tiles 4c..4c+3
NCH = N // CH


def build_kernel(ctx, tc, out_d, x_d, s_d, cj_d, rib_d, w_d, w2b_d):
    nc = tc.nc

    singles = ctx.enter_context(tc.tile_pool(name="singles", bufs=1))
    xt_pool = ctx.enter_context(tc.tile_pool(name="xt", bufs=4))
    g0_pool = ctx.enter_context(tc.tile_pool(name="g0", bufs=3))
    sqd_pool = ctx.enter_context(tc.tile_pool(name="sqd", bufs=3))
    y0t_pool = ctx.enter_context(tc.tile_pool(name="y0t", bufs=4))
    osb_pool = ctx.enter_context(tc.tile_pool(name="osb", bufs=4))
    sm_pool = ctx.enter_context(tc.tile_pool(name="sm", bufs=4))
    macc_pool = ctx.enter_context(tc.tile_pool(name="macc", bufs=2))
    ps_tr = ctx.enter_context(tc.tile_pool(name="ps_tr", bufs=2, space="PSUM"))
    ps_g = ctx.enter_context(tc.tile_pool(name="ps_g", bufs=2, space="PSUM"))
    ps_y = ctx.enter_context(tc.tile_pool(name="ps_y", bufs=1, space="PSUM"))

    # --- persistent SBUF tensors ---
    st = singles.tile([128, KT * N], F32R)            # S^T: [k*N + j] layout
    w_sb = singles.tile([128, KT * NOUT], F32R)       # W c-tiles
    cj_sb = singles.tile([128, NJT], F32)             # cj[t*128+p] at [p, t]
    rib_sb = singles.tile([128, R], F32)              # ri/2 broadcast across partitions
    w2b_sb = singles.tile([128, NOUT], F32)           # w2 broadcast across partitions
    ident = singles.tile([128, 128], F32)

    make_identity(nc, ident[:])

    def load_st_chunk(c):
        for k in range(KT):
            nc.sync.dma_start(
                out=st[:, k * N + c * CH: k * N + (c + 1) * CH],
                in_=s_d[bass.ts(k, 128), c * CH:(c + 1) * CH].bitcast(F32R),
            )

    nc.sync.dma_start(out=rib_sb[:], in_=rib_d)
    load_st_chunk(0)
    nc.sync.dma_start(out=cj_sb[:], in_=cj_d)
    nc.sync.dma_start(out=w2b_sb[:], in_=w2b_d)

    # --- main: per i-block: gram strip -> G0 -> Y0T accum -> normalize -> GEMM ---
    for ib in range(NIB):
        icol0 = ib * IB  # local column offset into S^T / riB
        psy = [ps_y.tile([128, IB], F32, tag=f"y{c}", name=f"psy{c}")
               for c in range(KT)]
        macc = macc_pool.tile([128, IB], F32, tag="macc")

        for jt in range(NJT):
            xt = xt_pool.tile([128, D], F32R, tag="xt")
            nc.sync.dma_start(out=xt[:], in_=x_d[bass.ts(jt, 128), :].bitcast(F32R))

            if ib == 0:
                if jt == 0:
                    load_st_chunk(1)
                    load_st_chunk(2)
                elif jt % 4 == 0 and jt // 4 + 2 < NCH:
                    load_st_chunk(jt // 4 + 2)
                if jt == 32:
                    for kt in range(KT):
                        nc.sync.dma_start(
                            out=w_sb[:, kt * NOUT:(kt + 1) * NOUT],
                            in_=w_d[bass.ts(kt, 128), :].bitcast(F32R),
                        )

            psg = ps_g.tile([128, IB], F32, tag="g")
            for k in range(KT):
                nc.tensor.matmul(
                    psg[:],
                    st[:, k * N + jt * 128: k * N + jt * 128 + 128],
                    st[:, k * N + icol0: k * N + icol0 + IB],
                    start=(k == 0),
                    stop=(k == KT - 1),
                )
            # sqd = gram - ri/2  (broadcast tile; per-i term of the expansion);
            # PSUM -> SBUF, freeing the psg bank for the next gram group
            sqd = sqd_pool.tile([128, IB], F32, tag="sqd")
            nc.vector.tensor_sub(sqd[:], psg[:], rib_sb[:, icol0:icol0 + IB])

            # G0^T tile = sqrt(-2*sqd + cj[j])   (cj includes +CLAMP)
            g0 = g0_pool.tile([128, IB], F32R, tag="g0")
            nc.scalar.activation(
                out=g0[:], in_=sqd[:],
                func=mybir.ActivationFunctionType.Sqrt,
                bias=cj_sb[:, jt:jt + 1], scale=-2.0,
            )

            if jt == 0:
                nc.vector.tensor_copy(out=macc[:], in_=g0[:].bitcast(F32))
            else:
                nc.vector.tensor_max(macc[:], macc[:], g0[:].bitcast(F32))

            # software pipeline: issue Y0T matmuls one step behind the gram so
            # the PE fills the ACT sqrt latency with the next gram
            if jt > 0:
                pg0, pxt = prev
                for c in range(KT):
                    nc.tensor.matmul(
                        psy[c][:], pxt[:, bass.ts(c, 128)], pg0[:],
                        start=(jt == 1), stop=False,
                    )
            prev = (g0, xt)

        pg0, pxt = prev
        for c in range(KT):
            nc.tensor.matmul(
                psy[c][:], pxt[:, bass.ts(c, 128)], pg0[:],
                start=False, stop=True,
            )

        # tail, part 1: rowmax -> -1/rowmax per i sub-tile
        ninvs = []
        for s in range(NSUB):
            pst = ps_tr.tile([128, 128], F32, tag="tr")
            nc.tensor.transpose(pst[:], macc[:, bass.ts(s, 128)], ident[:])
            rm = sm_pool.tile([128, 1], F32, tag="rm")
            nc.vector.tensor_reduce(
                out=rm[:], in_=pst[:], axis=mybir.AxisListType.X,
                op=mybir.AluOpType.max,
            )
            nrm = sm_pool.tile([128, 1], F32, tag="nrm")
            nc.vector.tensor_scalar_mul(nrm[:], rm[:], -1.0)
            ninv = sm_pool.tile([128, 1], F32, tag="ninv", name=f"ninv{s}")
            nc.vector.reciprocal(ninv[:], nrm[:])  # -1/rowmax
            ninvs.append(ninv)

        # tail, part 2: Y0T out of PSUM, then GEMM with W; scale + w2 on output
        y0ts = []
        for c in range(KT):
            y0t = y0t_pool.tile([128, IB], F32R, tag="y0t", name=f"y0t{c}")
            if c % 2 == 0:
                nc.vector.tensor_copy(out=y0t[:], in_=psy[c][:])
            else:
                nc.scalar.copy(out=y0t[:], in_=psy[c][:])
            y0ts.append(y0t)

        for s in range(NSUB):
            pso = ps_g.tile([128, NOUT], F32, tag="g", name=f"pso{s}")
            for c in range(KT):
                nc.tensor.matmul(
                    pso[:],
                    y0ts[c][:, bass.ts(s, 128)],
                    w_sb[:, c * NOUT:(c + 1) * NOUT],
                    start=(c == 0),
                    stop=(c == KT - 1),
                )
            acted = osb_pool.tile([128, NOUT], F32, tag="osb", name=f"acted{s}")
            nc.scalar.activation(
                out=acted[:], in_=pso[:],
                func=mybir.ActivationFunctionType.Copy, scale=ninvs[s][:],
            )
            osb = osb_pool.tile([128, NOUT], F32, tag="osb", name=f"osb{s}")
            nc.vector.tensor_add(osb[:], acted[:], w2b_sb[:])
            nc.sync.dma_start(out=out_d[bass.ts(ib * NSUB + s, 128), :], in_=osb[:])


_NC_CACHE = {}


def _build_nc():
    if "nc" in _NC_CACHE:
        return _NC_CACHE["nc"]
    nc = bacc.Bacc("TRN2", target_bir_lowering=False, debug=False, num_devices=M)
    x_d = nc.dram_tensor("x", [N, D], F32, kind="ExternalInput").ap()
    s_d = nc.dram_tensor("simT", [D, N], F32, kind="ExternalInput").ap()
    cj_d = nc.dram_tensor("cj", [128, NJT], F32, kind="ExternalInput").ap()
    rib_d = nc.dram_tensor("riB", [128, R], F32, kind="ExternalInput").ap()
    w_d = nc.dram_tensor("w", [D, NOUT], F32, kind="ExternalInput").ap()
    w2b_d = nc.dram_tensor("w2B", [128, NOUT], F32, kind="ExternalInput").ap()
    out_d = nc.dram_tensor("out", [R, NOUT], F32, kind="ExternalOutput").ap()
    with tile.TileContext(nc) as tc, ExitStack() as ctx:
        build_kernel(ctx, tc, out_d, x_d, s_d, cj_d, rib_d, w_d, w2b_d)
    nc.compile()
    _NC_CACHE["nc"] = nc
    return nc


def make_in_maps(x, sim_feat, weight):
    x = np.ascontiguousarray(x, dtype=np.float32)
    sim = np.ascontiguousarray(sim_feat, dtype=np.float32)
    w = np.ascontiguousarray(weight, dtype=np.float32)

    sim64 = sim.astype(np.float64)
    sq = (sim64 * sim64).sum(1)
    ss = sim64.sum(1)
    cj_full = (sq - 2.0 * EPS * ss + CLAMP).astype(np.float32)         # [N]
    ri_full = sq + 2.0 * EPS * ss + D * EPS * EPS                      # [N] f64
    colsum = x.astype(np.float64).sum(0)
    w2 = (colsum @ w.astype(np.float64)).astype(np.float32)
    w2B = np.ascontiguousarray(np.broadcast_to(w2, (128, NOUT)))

    in_maps = []
    for c in range(M):
        shift = c * R
        sim_c = np.ascontiguousarray(np.roll(sim, -shift, axis=0).T)
        x_c = np.roll(x, -shift, axis=0)
        cj_c = np.ascontiguousarray(
            np.roll(cj_full, -shift).reshape(NJT, 128).T
        )                                                               # [128, NJT]
        ri_c = (ri_full[shift:shift + R] / 2.0).astype(np.float32)
        rib_c = np.ascontiguousarray(np.broadcast_to(ri_c, (128, R)))
        in_maps.append(
            {"x": x_c, "simT": sim_c, "cj": cj_c, "riB": rib_c,
             "w": w, "w2B": w2B}
        )
    return in_maps


def kernel(x, sim_feat, weight, _trace=False, **kw):
    nc = _build_nc()
    in_maps = make_in_maps(x, sim_feat, weight)
    res = run_bass_kernel_spmd(nc, in_maps, list(range(M)), trace=_trace, **kw)
    out = np.concatenate([res.results[c]["out"] for c in range(M)], axis=0)
    if _trace:
        return out, res
    return out


# revision 8
# speedup vs baseline: 1.3604x; 1.1833x over previous
"""Bass/Trainium2 kernel for nn_Graph_Layer (gnn_message_passing).

Reference math (N=8192, D=512):
    G0[i,j] = ||s_i - s_j + eps||_2   (pairwise distances, Gram trick)
    G = 1 - G0 / rowmax(G0)
    out = (G @ x) @ W

Decomposition (row-shard over 8 cores, 1024 rows each):
    sqd[i,j] = ri[i] + cj[j] - 2*gram[i,j]        (ri, cj host-precomputed)
    G0 = sqrt(sqd + CLAMP)                         (CLAMP covers tf32 noise on diag)
    rowmax[i] = max_j G0[i,j]
    (G @ x)[i,:] = colsum_x - Y0[i,:]/rowmax[i],   Y0 = G0 @ x
    out[i,:]  = w2 - (Y0 @ W)[i,:]/rowmax[i],      w2 = colsum_x @ W (host)

On device the distance strip is computed TRANSPOSED (sqd^T[j,i]) so G0 tiles
come out with j (the contraction dim of Y0) on partitions. The per-i "ri" term
is added by the Vector engine from a host-precomputed broadcast tile (riB)
instead of a 1-row PE matmul: a 1-row matmul costs the same PE cycles as a
full one (cost = output free size), and its LDWEIGHTS stalled the PE pipeline
every iteration, dropping the p-state.

Y0 is accumulated transposed (Y0T[c,i], stationary = x c-slices, moving = G0)
so no transposes are needed before the W GEMM; the GEMM output lands [i, n]
with i on partitions, where the -1/rowmax scale is a per-partition ACT scale
and w2 is a DVE add of a host broadcast tile.

Each core sees its own np.roll'ed copy of the inputs so local rows are always
[0,1024): a single uniform SPMD program runs on all 8 cores. All matmuls use
float32r (TF32 mode).
"""

import numpy as np
from contextlib import ExitStack

import concourse.bass as bass
from concourse import bacc
import concourse.tile as tile
from concourse import mybir
from concourse.bass_utils import run_bass_kernel_spmd
from concourse.masks import make_identity

N, D, NOUT = 8192, 512, 512
M = 8                 # cores
R = N // M            # 1024 local rows per core
EPS = 1e-6
CLAMP = 0.3           # covers tf32 rounding noise on the diagonal; ~1e-4 rel effect off-diag
F32 = mybir.dt.float32
F32R = mybir.dt.float32r

KT = D // 128         # 4 contraction sub-tiles
NJT = N // 128        # 64 j tiles
IB = 512              # i block (free dim of the gram matmuls)
NIB = R // IB         # 2
NSUB = IB // 128      # 4 sub-tiles of 128 rows per i block

CH = 512              # S^T DMA chunk width (columns); chunk c covers j_tiles 4c..4c+3
NCH = N // CH


def build_kernel(ctx, tc, out_d, x_d, s_d, cj_d, rib_d, w_d, w2b_d):
    nc = tc.nc

    singles = ctx.enter_context(tc.tile_pool(name="singles", bufs=1))
    xt_pool = ctx.enter_context(tc.tile_pool(name="xt", bufs=4))
    g0_pool = ctx.enter_context(tc.tile_pool(name="g0", bufs=3))
    sqd_pool = ctx.enter_context(tc.tile_pool(name="sqd", bufs=3))
    y0t_pool = ctx.enter_context(tc.tile_pool(name="y0t", bufs=4))
    osb_pool = ctx.enter_context(tc.tile_pool(name="osb", bufs=4))
    sm_pool = ctx.enter_context(tc.tile_pool(name="sm", bufs=4))
    macc_pool = ctx.enter_context(tc.tile_pool(name="macc", bufs=2))
    ps_tr = ctx.enter_context(tc.tile_pool(name="ps_tr", bufs=2, space="PSUM"))
    ps_g = ctx.enter_context(tc.tile_pool(name="ps_g", bufs=2, space="PSUM"))
    ps_y = ctx.enter_context(tc.tile_pool(name="ps_y", bufs=1, space="PSUM"))

    # --- persistent SBUF tensors ---
    st = singles.tile([128, KT * N], F32R)            # S^T: [k*N + j] layout
    w_sb = singles.tile([128, KT * NOUT], F32R)       # W c-tiles
    cj_sb = singles.tile([128, NJT], F32)             # cj[t*128+p] at [p, t]
    rib_sb = singles.tile([128, R], F32)              # ri/2 broadcast across partitions
    w2b_sb = singles.tile([128, NOUT], F32)           # w2 broadcast across partitions
    ident = singles.tile([128, 128], F32)

    make_identity(nc, ident[:])

    def load_st_chunk(c):
        for k in range(KT):
            nc.sync.dma_start(
                out=st[:, k * N + c * CH: k * N + (c + 1) * CH],
                in_=s_d[bass.ts(k, 128), c * CH:(c + 1) * CH].bitcast(F32R),
            )

    nc.sync.dma_start(out=rib_sb[:], in_=rib_d)
    load_st_chunk(0)
    nc.sync.dma_start(out=cj_sb[:], in_=cj_d)
    nc.sync.dma_start(out=w2b_sb[:], in_=w2b_d)

    # --- main: per i-block: gram strip -> G0 -> Y0T accum -> normalize -> GEMM ---
    for ib in range(NIB):
        icol0 = ib * IB  # local column offset into S^T / riB
        psy = [ps_y.tile([128, IB], F32, tag=f"y{c}", name=f"psy{c}")
               for c in range(KT)]
        macc = macc_pool.tile([128, IB], F32, tag="macc")

        for jt in range(NJT):
            xt = xt_pool.tile([128, D], F32R, tag="xt")
            nc.sync.dma_start(out=xt[:], in_=x_d[bass.ts(jt, 128), :].bitcast(F32R))

            if ib == 0:
                if jt == 0:
                    load_st_chunk(1)
                    load_st_chunk(2)
                elif jt % 4 == 0 and jt // 4 + 2 < NCH:
                    load_st_chunk(jt // 4 + 2)
                if jt == 32:
                    for kt in range(KT):
                        nc.sync.dma_start(
                            out=w_sb[:, kt * NOUT:(kt + 1) * NOUT],
                            in_=w_d[bass.ts(kt, 128), :].bitcast(F32R),
                        )

            psg = ps_g.tile([128, IB], F32, tag="g")
            for k in range(KT):
                nc.tensor.matmul(
                    psg[:],
                    st[:, k * N + jt * 128: k * N + jt * 128 + 128],
                    st[:, k * N + icol0: k * N + icol0 + IB],
                    start=(k == 0),
                    stop=(k == KT - 1),
                )
            # sqd = gram - ri/2  (broadcast tile; per-i term of the expansion);
            # PSUM -> SBUF, freeing the psg bank for the next gram group
            sqd = sqd_pool.tile([128, IB], F32, tag="sqd")
            nc.vector.tensor_sub(sqd[:], psg[:], rib_sb[:, icol0:icol0 + IB])

            # G0^T tile = sqrt(-2*sqd + cj[j])   (cj includes +CLAMP)
            g0 = g0_pool.tile([128, IB], F32R, tag="g0")
            nc.scalar.activation(
                out=g0[:], in_=sqd[:],
                func=mybir.ActivationFunctionType.Sqrt,
                bias=cj_sb[:, jt:jt + 1], scale=-2.0,
            )

            # software pipeline: the Y0T matmuls AND the rowmax update run one
            # step behind the gram. For the DVE this keeps tensor_sub (which
            # releases the psg PSUM slot) from queueing behind tensor_max
            # (which waits on the ACT sqrt) — otherwise the psg recycle stalls
            # the PE every other iteration.
            if jt > 0:
                pg0, pxt = prev
                if jt == 1:
                    nc.vector.tensor_copy(out=macc[:], in_=pg0[:].bitcast(F32))
                else:
                    nc.vector.tensor_max(macc[:], macc[:], pg0[:].bitcast(F32))
                for c in range(KT):
                    nc.tensor.matmul(
                        psy[c][:], pxt[:, bass.ts(c, 128)], pg0[:],
                        start=(jt == 1), stop=False,
                    )
            prev = (g0, xt)

        pg0, pxt = prev
        nc.vector.tensor_max(macc[:], macc[:], pg0[:].bitcast(F32))
        for c in range(KT):
            nc.tensor.matmul(
                psy[c][:], pxt[:, bass.ts(c, 128)], pg0[:],
                start=False, stop=True,
            )

        # tail, part 1: rowmax -> -1/rowmax per i sub-tile
        ninvs = []
        for s in range(NSUB):
            pst = ps_tr.tile([128, 128], F32, tag="tr")
            nc.tensor.transpose(pst[:], macc[:, bass.ts(s, 128)], ident[:])
            rm = sm_pool.tile([128, 1], F32, tag="rm")
            nc.vector.tensor_reduce(
                out=rm[:], in_=pst[:], axis=mybir.AxisListType.X,
                op=mybir.AluOpType.max,
            )
            nrm = sm_pool.tile([128, 1], F32, tag="nrm")
            nc.vector.tensor_scalar_mul(nrm[:], rm[:], -1.0)
            ninv = sm_pool.tile([128, 1], F32, tag="ninv", name=f"ninv{s}")
            nc.vector.reciprocal(ninv[:], nrm[:])  # -1/rowmax
            ninvs.append(ninv)

        # tail, part 2: Y0T out of PSUM, then GEMM with W; scale + w2 on output
        y0ts = []
        for c in range(KT):
            y0t = y0t_pool.tile([128, IB], F32R, tag="y0t", name=f"y0t{c}")
            if c % 2 == 0:
                nc.vector.tensor_copy(out=y0t[:], in_=psy[c][:])
            else:
                nc.scalar.copy(out=y0t[:], in_=psy[c][:])
            y0ts.append(y0t)

        for s in range(NSUB):
            pso = ps_g.tile([128, NOUT], F32, tag="g", name=f"pso{s}")
            for c in range(KT):
                nc.tensor.matmul(
                    pso[:],
                    y0ts[c][:, bass.ts(s, 128)],
                    w_sb[:, c * NOUT:(c + 1) * NOUT],
                    start=(c == 0),
                    stop=(c == KT - 1),
                )
            acted = osb_pool.tile([128, NOUT], F32, tag="osb", name=f"acted{s}")
            nc.scalar.activation(
                out=acted[:], in_=pso[:],
                func=mybir.ActivationFunctionType.Copy, scale=ninvs[s][:],
            )
            osb = osb_pool.tile([128, NOUT], F32, tag="osb", name=f"osb{s}")
            nc.vector.tensor_add(osb[:], acted[:], w2b_sb[:])
            nc.sync.dma_start(out=out_d[bass.ts(ib * NSUB + s, 128), :], in_=osb[:])


_NC_CACHE = {}


def _build_nc():
    if "nc" in _NC_CACHE:
        return _NC_CACHE["nc"]
    nc = bacc.Bacc("TRN2", target_bir_lowering=False, debug=False, num_devices=M)
    x_d = nc.dram_tensor("x", [N, D], F32, kind="ExternalInput").ap()
    s_d = nc.dram_tensor("simT", [D, N], F32, kind="ExternalInput").ap()
    cj_d = nc.dram_tensor("cj", [128, NJT], F32, kind="ExternalInput").ap()
    rib_d = nc.dram_tensor("riB", [128, R], F32, kind="ExternalInput").ap()
    w_d = nc.dram_tensor("w", [D, NOUT], F32, kind="ExternalInput").ap()
    w2b_d = nc.dram_tensor("w2B", [128, NOUT], F32, kind="ExternalInput").ap()
    out_d = nc.dram_tensor("out", [R, NOUT], F32, kind="ExternalOutput").ap()
    with tile.TileContext(nc) as tc, ExitStack() as ctx:
        build_kernel(ctx, tc, out_d, x_d, s_d, cj_d, rib_d, w_d, w2b_d)
    nc.compile()
    _NC_CACHE["nc"] = nc
    return nc


def make_in_maps(x, sim_feat, weight):
    x = np.ascontiguousarray(x, dtype=np.float32)
    sim = np.ascontiguousarray(sim_feat, dtype=np.float32)
    w = np.ascontiguousarray(weight, dtype=np.float32)

    sim64 = sim.astype(np.float64)
    sq = (sim64 * sim64).sum(1)
    ss = sim64.sum(1)
    cj_full = (sq - 2.0 * EPS * ss + CLAMP).astype(np.float32)         # [N]
    ri_full = sq + 2.0 * EPS * ss + D * EPS * EPS                      # [N] f64
    colsum = x.astype(np.float64).sum(0)
    w2 = (colsum @ w.astype(np.float64)).astype(np.float32)
    w2B = np.ascontiguousarray(np.broadcast_to(w2, (128, NOUT)))

    in_maps = []
    for c in range(M):
        shift = c * R
        sim_c = np.ascontiguousarray(np.roll(sim, -shift, axis=0).T)
        x_c = np.roll(x, -shift, axis=0)
        cj_c = np.ascontiguousarray(
            np.roll(cj_full, -shift).reshape(NJT, 128).T
        )                                                               # [128, NJT]
        ri_c = (ri_full[shift:shift + R] / 2.0).astype(np.float32)
        rib_c = np.ascontiguousarray(np.broadcast_to(ri_c, (128, R)))
        in_maps.append(
            {"x": x_c, "simT": sim_c, "cj": cj_c, "riB": rib_c,
             "w": w, "w2B": w2B}
        )
    return in_maps


def kernel(x, sim_feat, weight, _trace=False, **kw):
    nc = _build_nc()
    in_maps = make_in_maps(x, sim_feat, weight)
    res = run_bass_kernel_spmd(nc, in_maps, list(range(M)), trace=_trace, **kw)
    out = np.concatenate([res.results[c]["out"] for c in range(M)], axis=0)
    if _trace:
        return out, res
    return out


# revision 16
# speedup vs baseline: 1.3709x; 1.0078x over previous
"""Bass/Trainium2 kernel for nn_Graph_Layer (gnn_message_passing).

Reference math (N=8192, D=512):
    G0[i,j] = ||s_i - s_j + eps||_2   (pairwise distances, Gram trick)
    G = 1 - G0 / rowmax(G0)
    out = (G @ x) @ W

Decomposition (row-shard over 8 cores, 1024 rows each):
    sqd[i,j] = ri[i] + cj[j] - 2*gram[i,j]        (ri, cj host-precomputed)
    G0 = sqrt(sqd + CLAMP)                         (CLAMP covers tf32 noise on diag)
    rowmax[i] = max_j G0[i,j]
    (G @ x)[i,:] = colsum_x - Y0[i,:]/rowmax[i],   Y0 = G0 @ x
    out[i,:]  = w2 - (Y0 @ W)[i,:]/rowmax[i],      w2 = colsum_x @ W (host)

On device the distance strip is computed TRANSPOSED (sqd^T[j,i]) so G0 tiles
come out with j (the contraction dim of Y0) on partitions. The per-i "ri" term
is added by the Vector engine from a host-precomputed broadcast tile (riB)
instead of a 1-row PE matmul: a 1-row matmul costs the same PE cycles as a
full one (cost = output free size), and its LDWEIGHTS stalled the PE pipeline
every iteration, dropping the p-state.

Y0 is accumulated transposed (Y0T[c,i], stationary = x c-slices, moving = G0)
so no transposes are needed before the W GEMM; the GEMM output lands [i, n]
with i on partitions, where the -1/rowmax scale is a per-partition ACT scale
and w2 is a DVE add of a host broadcast tile.

Each core sees its own np.roll'ed copy of the inputs so local rows are always
[0,1024): a single uniform SPMD program runs on all 8 cores. All matmuls use
float32r (TF32 mode).
"""

import numpy as np
import ml_dtypes
from contextlib import ExitStack

import concourse.bass as bass
from concourse import bacc
import concourse.tile as tile
from concourse import mybir
from concourse.bass_utils import run_bass_kernel_spmd
from concourse.masks import make_identity

N, D, NOUT = 8192, 512, 512
M = 8                 # cores
R = N // M            # 1024 local rows per core
EPS = 1e-6
CLAMP = 0.3           # covers tf32 rounding noise on the diagonal; ~1e-4 rel effect off-diag
F32 = mybir.dt.float32
F32R = mybir.dt.float32r
BF16 = mybir.dt.bfloat16

KT = D // 128         # 4 contraction sub-tiles
NJT = N // 128        # 64 j tiles
IB = 512              # i block (free dim of the gram matmuls)
NIB = R // IB         # 2
NSUB = IB // 128      # 4 sub-tiles of 128 rows per i block

CH = 512              # S^T DMA chunk width (columns); chunk c covers j_tiles 4c..4c+3
NCH = N // CH


def build_kernel(ctx, tc, out_d, x_d, s_d, cj_d, rib_d, w_d, w2b_d):
    nc = tc.nc

    singles = ctx.enter_context(tc.tile_pool(name="singles", bufs=1))
    xt_pool = ctx.enter_context(tc.tile_pool(name="xt", bufs=4))
    g0_pool = ctx.enter_context(tc.tile_pool(name="g0", bufs=3))
    sqd_pool = ctx.enter_context(tc.tile_pool(name="sqd", bufs=3))
    y0t_pool = ctx.enter_context(tc.tile_pool(name="y0t", bufs=4))
    osb_pool = ctx.enter_context(tc.tile_pool(name="osb", bufs=4))
    sm_pool = ctx.enter_context(tc.tile_pool(name="sm", bufs=4))
    macc_pool = ctx.enter_context(tc.tile_pool(name="macc", bufs=2))
    ps_tr = ctx.enter_context(tc.tile_pool(name="ps_tr", bufs=2, space="PSUM"))
    ps_g = ctx.enter_context(tc.tile_pool(name="ps_g", bufs=2, space="PSUM"))
    ps_y = ctx.enter_context(tc.tile_pool(name="ps_y", bufs=1, space="PSUM"))

    # --- persistent SBUF tensors ---
    st = singles.tile([128, KT * N], BF16)            # S^T: [k*N + j] layout
    w_sb = singles.tile([128, KT * NOUT], F32R)       # W c-tiles
    cj_sb = singles.tile([128, NJT], F32)             # cj[t*128+p] at [p, t]
    rib_sb = singles.tile([128, R], F32)              # ri/2 broadcast across partitions
    w2b_sb = singles.tile([128, NOUT], F32)           # w2 broadcast across partitions
    ident = singles.tile([128, 128], F32)

    make_identity(nc, ident[:])

    def load_st_chunk(c):
        for k in range(KT):
            nc.sync.dma_start(
                out=st[:, k * N + c * CH: k * N + (c + 1) * CH],
                in_=s_d[bass.ts(k, 128), c * CH:(c + 1) * CH],
            )

    # chunk 0 first: the first gram matmuls are gated on it; riB isn't needed
    # until the first DVE sub, cj until the first ACT, w2B until the tail
    load_st_chunk(0)
    nc.sync.dma_start(out=rib_sb[:], in_=rib_d)
    nc.sync.dma_start(out=cj_sb[:], in_=cj_d)
    nc.sync.dma_start(out=w2b_sb[:], in_=w2b_d)

    # --- main: per i-block: gram strip -> G0 -> Y0T accum -> normalize -> GEMM ---
    for ib in range(NIB):
        icol0 = ib * IB  # local column offset into S^T / riB
        psy = [ps_y.tile([128, IB], F32, tag=f"y{c}", name=f"psy{c}")
               for c in range(KT)]
        macc = macc_pool.tile([128, IB], F32, tag="macc")

        for jt in range(NJT):
            xt = xt_pool.tile([128, D], F32R, tag="xt")
            nc.sync.dma_start(out=xt[:], in_=x_d[bass.ts(jt, 128), :].bitcast(F32R))

            if ib == 0:
                if jt == 0:
                    load_st_chunk(1)
                    load_st_chunk(2)
                elif jt % 4 == 0 and jt // 4 + 2 < NCH:
                    load_st_chunk(jt // 4 + 2)
                if jt == 32:
                    for kt in range(KT):
                        nc.sync.dma_start(
                            out=w_sb[:, kt * NOUT:(kt + 1) * NOUT],
                            in_=w_d[bass.ts(kt, 128), :].bitcast(F32R),
                        )

            psg = ps_g.tile([128, IB], F32, tag="g")
            for k in range(KT):
                nc.tensor.matmul(
                    psg[:],
                    st[:, k * N + jt * 128: k * N + jt * 128 + 128],
                    st[:, k * N + icol0: k * N + icol0 + IB],
                    start=(k == 0),
                    stop=(k == KT - 1),
                )
            # sqd = gram - ri/2  (broadcast tile; per-i term of the expansion);
            # PSUM -> SBUF, freeing the psg bank for the next gram group
            sqd = sqd_pool.tile([128, IB], F32, tag="sqd")
            nc.vector.tensor_sub(sqd[:], psg[:], rib_sb[:, icol0:icol0 + IB])

            # G0^T tile = sqrt(-2*sqd + cj[j])   (cj includes +CLAMP)
            g0 = g0_pool.tile([128, IB], F32R, tag="g0")
            nc.scalar.activation(
                out=g0[:], in_=sqd[:],
                func=mybir.ActivationFunctionType.Sqrt,
                bias=cj_sb[:, jt:jt + 1], scale=-2.0,
            )

            # software pipeline: the Y0T matmuls AND the rowmax update run one
            # step behind the gram. For the DVE this keeps tensor_sub (which
            # releases the psg PSUM slot) from queueing behind tensor_max
            # (which waits on the ACT sqrt) — otherwise the psg recycle stalls
            # the PE every other iteration.
            if jt > 0:
                pg0, pxt = prev
                if jt == 1:
                    nc.vector.tensor_copy(out=macc[:], in_=pg0[:].bitcast(F32))
                else:
                    nc.vector.tensor_max(macc[:], macc[:], pg0[:].bitcast(F32))
                for c in range(KT):
                    nc.tensor.matmul(
                        psy[c][:], pxt[:, bass.ts(c, 128)], pg0[:],
                        start=(jt == 1), stop=False,
                    )
            prev = (g0, xt)

        pg0, pxt = prev
        nc.vector.tensor_max(macc[:], macc[:], pg0[:].bitcast(F32))
        for c in range(KT):
            nc.tensor.matmul(
                psy[c][:], pxt[:, bass.ts(c, 128)], pg0[:],
                start=False, stop=True,
            )

        # tail, part 1: rowmax -> -1/rowmax per i sub-tile
        ninvs = []
        for s in range(NSUB):
            pst = ps_tr.tile([128, 128], F32, tag="tr")
            nc.tensor.transpose(pst[:], macc[:, bass.ts(s, 128)], ident[:])
            rm = sm_pool.tile([128, 1], F32, tag="rm")
            nc.vector.tensor_reduce(
                out=rm[:], in_=pst[:], axis=mybir.AxisListType.X,
                op=mybir.AluOpType.max,
            )
            nrm = sm_pool.tile([128, 1], F32, tag="nrm")
            nc.vector.tensor_scalar_mul(nrm[:], rm[:], -1.0)
            ninv = sm_pool.tile([128, 1], F32, tag="ninv", name=f"ninv{s}")
            nc.vector.reciprocal(ninv[:], nrm[:])  # -1/rowmax
            ninvs.append(ninv)

        # tail, part 2: Y0T out of PSUM, then GEMM with W; scale + w2 on output
        y0ts = []
        for c in range(KT):
            y0t = y0t_pool.tile([128, IB], F32R, tag="y0t", name=f"y0t{c}")
            if c % 2 == 0:
                nc.vector.tensor_copy(out=y0t[:], in_=psy[c][:])
            else:
                nc.scalar.copy(out=y0t[:], in_=psy[c][:])
            y0ts.append(y0t)

        for s in range(NSUB):
            pso = ps_g.tile([128, NOUT], F32, tag="g", name=f"pso{s}")
            for c in range(KT):
                nc.tensor.matmul(
                    pso[:],
                    y0ts[c][:, bass.ts(s, 128)],
                    w_sb[:, c * NOUT:(c + 1) * NOUT],
                    start=(c == 0),
                    stop=(c == KT - 1),
                )
            acted = osb_pool.tile([128, NOUT], F32, tag="osb", name=f"acted{s}")
            nc.scalar.activation(
                out=acted[:], in_=pso[:],
                func=mybir.ActivationFunctionType.Copy, scale=ninvs[s][:],
            )
            osb = osb_pool.tile([128, NOUT], F32, tag="osb", name=f"osb{s}")
            nc.vector.tensor_add(osb[:], acted[:], w2b_sb[:])
            nc.sync.dma_start(out=out_d[bass.ts(ib * NSUB + s, 128), :], in_=osb[:])


_NC_CACHE = {}


def _build_nc():
    if "nc" in _NC_CACHE:
        return _NC_CACHE["nc"]
    nc = bacc.Bacc("TRN2", target_bir_lowering=False, debug=False, num_devices=M)
    x_d = nc.dram_tensor("x", [N, D], F32, kind="ExternalInput").ap()
    s_d = nc.dram_tensor("simT", [D, N], BF16, kind="ExternalInput").ap()
    cj_d = nc.dram_tensor("cj", [128, NJT], F32, kind="ExternalInput").ap()
    rib_d = nc.dram_tensor("riB", [128, R], F32, kind="ExternalInput").ap()
    w_d = nc.dram_tensor("w", [D, NOUT], F32, kind="ExternalInput").ap()
    w2b_d = nc.dram_tensor("w2B", [128, NOUT], F32, kind="ExternalInput").ap()
    out_d = nc.dram_tensor("out", [R, NOUT], F32, kind="ExternalOutput").ap()
    with tile.TileContext(nc) as tc, ExitStack() as ctx:
        build_kernel(ctx, tc, out_d, x_d, s_d, cj_d, rib_d, w_d, w2b_d)
    nc.compile()
    _NC_CACHE["nc"] = nc
    return nc


def make_in_maps(x, sim_feat, weight):
    x = np.ascontiguousarray(x, dtype=np.float32)
    w = np.ascontiguousarray(weight, dtype=np.float32)
    # the gram matmuls run in bf16: round sim on the host and derive ri/cj
    # from the ROUNDED values so the device computes exact distances of the
    # rounded vectors (error = distance perturbation only, ~1e-3 relative)
    sim_bf = np.asarray(sim_feat, dtype=np.float32).astype(ml_dtypes.bfloat16)

    sim64 = sim_bf.astype(np.float64)
    sq = (sim64 * sim64).sum(1)
    ss = sim64.sum(1)
    cj_full = (sq - 2.0 * EPS * ss + CLAMP).astype(np.float32)         # [N]
    ri_full = sq + 2.0 * EPS * ss + D * EPS * EPS                      # [N] f64
    colsum = x.astype(np.float64).sum(0)
    w2 = (colsum @ w.astype(np.float64)).astype(np.float32)
    w2B = np.ascontiguousarray(np.broadcast_to(w2, (128, NOUT)))

    in_maps = []
    for c in range(M):
        shift = c * R
        sim_c = np.ascontiguousarray(np.roll(sim_bf, -shift, axis=0).T)
        x_c = np.roll(x, -shift, axis=0)
        cj_c = np.ascontiguousarray(
            np.roll(cj_full, -shift).reshape(NJT, 128).T
        )                                                               # [128, NJT]
        ri_c = (ri_full[shift:shift + R] / 2.0).astype(np.float32)
        rib_c = np.ascontiguousarray(np.broadcast_to(ri_c, (128, R)))
        in_maps.append(
            {"x": x_c, "simT": sim_c, "cj": cj_c, "riB": rib_c,
             "w": w, "w2B": w2B}
        )
    return in_maps


def kernel(x, sim_feat, weight, _trace=False, **kw):
    nc = _build_nc()
    in_maps = make_in_maps(x, sim_feat, weight)
    res = run_bass_kernel_spmd(nc, in_maps, list(range(M)), trace=_trace, **kw)
    out = np.concatenate([res.results[c]["out"] for c in range(M)], axis=0)
    if _trace:
        return out, res
    return out


# revision 20
# speedup vs baseline: 1.4010x; 1.0219x over previous
"""Bass/Trainium2 kernel for nn_Graph_Layer (gnn_message_passing).

Reference math (N=8192, D=512):
    G0[i,j] = ||s_i - s_j + eps||_2   (pairwise distances, Gram trick)
    G = 1 - G0 / rowmax(G0)
    out = (G @ x) @ W

Decomposition (row-shard over 8 cores, 1024 rows each):
    sqd[i,j] = ri[i] + cj[j] - 2*gram[i,j]        (ri, cj host-precomputed)
    G0 = sqrt(sqd + CLAMP)                         (CLAMP covers tf32 noise on diag)
    rowmax[i] = max_j G0[i,j]
    (G @ x)[i,:] = colsum_x - Y0[i,:]/rowmax[i],   Y0 = G0 @ x
    out[i,:]  = w2 - (Y0 @ W)[i,:]/rowmax[i],      w2 = colsum_x @ W (host)

On device the distance strip is computed TRANSPOSED (sqd^T[j,i]) so G0 tiles
come out with j (the contraction dim of Y0) on partitions. The per-i "ri" term
is added by the Vector engine from a host-precomputed broadcast tile (riB)
instead of a 1-row PE matmul: a 1-row matmul costs the same PE cycles as a
full one (cost = output free size), and its LDWEIGHTS stalled the PE pipeline
every iteration, dropping the p-state.

Y0 is accumulated transposed (Y0T[c,i], stationary = x c-slices, moving = G0)
so no transposes are needed before the W GEMM; the GEMM output lands [i, n]
with i on partitions, where the -1/rowmax scale is a per-partition ACT scale
and w2 is a DVE add of a host broadcast tile.

Each core sees its own np.roll'ed copy of the inputs so local rows are always
[0,1024): a single uniform SPMD program runs on all 8 cores. All matmuls use
float32r (TF32 mode).
"""

import numpy as np
import ml_dtypes
from contextlib import ExitStack

import concourse.bass as bass
from concourse import bacc
import concourse.tile as tile
from concourse import mybir
from concourse.bass_utils import run_bass_kernel_spmd
from concourse.masks import make_identity

N, D, NOUT = 8192, 512, 512
M = 8                 # cores
R = N // M            # 1024 local rows per core
EPS = 1e-6
CLAMP = 0.3           # covers tf32 rounding noise on the diagonal; ~1e-4 rel effect off-diag
F32 = mybir.dt.float32
F32R = mybir.dt.float32r
BF16 = mybir.dt.bfloat16

KT = D // 128         # 4 contraction sub-tiles
NJT = N // 128        # 64 j tiles
IB = 512              # i block (free dim of the gram matmuls)
NIB = R // IB         # 2
NSUB = IB // 128      # 4 sub-tiles of 128 rows per i block

CH = 512              # S^T DMA chunk width (columns); chunk c covers j_tiles 4c..4c+3
NCH = N // CH


def build_kernel(ctx, tc, out_d, x_d, s_d, cj_d, rib_d, w_d, w2b_d):
    nc = tc.nc

    singles = ctx.enter_context(tc.tile_pool(name="singles", bufs=1))
    xt_pool = ctx.enter_context(tc.tile_pool(name="xt", bufs=4))
    g0_pool = ctx.enter_context(tc.tile_pool(name="g0", bufs=3))
    sqd_pool = ctx.enter_context(tc.tile_pool(name="sqd", bufs=3))
    y0t_pool = ctx.enter_context(tc.tile_pool(name="y0t", bufs=4))
    osb_pool = ctx.enter_context(tc.tile_pool(name="osb", bufs=4))
    sm_pool = ctx.enter_context(tc.tile_pool(name="sm", bufs=4))
    macc_pool = ctx.enter_context(tc.tile_pool(name="macc", bufs=2))
    ps_tr = ctx.enter_context(tc.tile_pool(name="ps_tr", bufs=1, space="PSUM"))
    ps_g = ctx.enter_context(tc.tile_pool(name="ps_g", bufs=2, space="PSUM"))
    ps_y = ctx.enter_context(tc.tile_pool(name="ps_y", bufs=1, space="PSUM"))
    ps_o = ctx.enter_context(tc.tile_pool(name="ps_o", bufs=1, space="PSUM"))

    # --- persistent SBUF tensors ---
    st = singles.tile([128, KT * N], BF16)            # S^T: [k*N + j] layout
    w_sb = singles.tile([128, KT * NOUT], F32R)       # W c-tiles
    cj_sb = singles.tile([128, NJT], F32)             # cj[t*128+p] at [p, t]
    rib_sb = singles.tile([128, R], F32)              # ri/2 broadcast across partitions
    w2b_sb = singles.tile([128, NOUT], F32)           # w2 broadcast across partitions
    ident = singles.tile([128, 128], F32)

    make_identity(nc, ident[:])

    def load_st_chunk(c):
        for k in range(KT):
            nc.sync.dma_start(
                out=st[:, k * N + c * CH: k * N + (c + 1) * CH],
                in_=s_d[bass.ts(k, 128), c * CH:(c + 1) * CH],
            )

    # chunk 0 first: the first gram matmuls are gated on it; riB isn't needed
    # until the first DVE sub, cj until the first ACT, w2B until the tail
    load_st_chunk(0)
    nc.sync.dma_start(out=rib_sb[:], in_=rib_d)
    nc.sync.dma_start(out=cj_sb[:], in_=cj_d)
    nc.sync.dma_start(out=w2b_sb[:], in_=w2b_d)

    def emit_out(ib, s, y0ts, ninv):
        """GEMM + scale + w2 + store for one 128-row output sub-tile."""
        pso = ps_o.tile([128, NOUT], F32, tag="o", name=f"pso{ib}_{s}")
        for c in range(KT):
            nc.tensor.matmul(
                pso[:],
                y0ts[c][:, bass.ts(s, 128)],
                w_sb[:, c * NOUT:(c + 1) * NOUT],
                start=(c == 0),
                stop=(c == KT - 1),
            )
        acted = osb_pool.tile([128, NOUT], F32, tag="osb", name=f"acted{ib}_{s}")
        nc.scalar.activation(
            out=acted[:], in_=pso[:],
            func=mybir.ActivationFunctionType.Copy, scale=ninv[:],
        )
        osb = osb_pool.tile([128, NOUT], F32, tag="osb", name=f"osb{ib}_{s}")
        nc.vector.tensor_add(osb[:], acted[:], w2b_sb[:])
        nc.sync.dma_start(out=out_d[bass.ts(ib * NSUB + s, 128), :], in_=osb[:])

    # --- main: per i-block: gram strip -> G0 -> Y0T accum -> normalize -> GEMM ---
    deferred = []  # output-side tail closures from the previous i-block
    for ib in range(NIB):
        icol0 = ib * IB  # local column offset into S^T / riB
        psy = [ps_y.tile([128, IB], F32, tag=f"y{c}", name=f"psy{c}")
               for c in range(KT)]
        macc = macc_pool.tile([128, IB], F32, tag="macc")

        for jt in range(NJT):
            # interleave the previous i-block's output tail into this block's
            # stream so its dependency latency hides under main-loop compute
            if deferred and jt >= 2 and jt % 2 == 0:
                deferred.pop(0)()
            xt = xt_pool.tile([128, D], F32R, tag="xt")
            nc.sync.dma_start(out=xt[:], in_=x_d[bass.ts(jt, 128), :].bitcast(F32R))

            if ib == 0:
                if jt == 0:
                    load_st_chunk(1)
                    load_st_chunk(2)
                elif jt % 4 == 0 and jt // 4 + 2 < NCH:
                    load_st_chunk(jt // 4 + 2)
                if jt == 32:
                    for kt in range(KT):
                        nc.sync.dma_start(
                            out=w_sb[:, kt * NOUT:(kt + 1) * NOUT],
                            in_=w_d[bass.ts(kt, 128), :].bitcast(F32R),
                        )

            psg = ps_g.tile([128, IB], F32, tag="g")
            for k in range(KT):
                nc.tensor.matmul(
                    psg[:],
                    st[:, k * N + jt * 128: k * N + jt * 128 + 128],
                    st[:, k * N + icol0: k * N + icol0 + IB],
                    start=(k == 0),
                    stop=(k == KT - 1),
                )
            # sqd = gram - ri/2  (broadcast tile; per-i term of the expansion);
            # PSUM -> SBUF, freeing the psg bank for the next gram group
            sqd = sqd_pool.tile([128, IB], F32, tag="sqd")
            nc.vector.tensor_sub(sqd[:], psg[:], rib_sb[:, icol0:icol0 + IB])

            # G0^T tile = sqrt(-2*sqd + cj[j])   (cj includes +CLAMP)
            g0 = g0_pool.tile([128, IB], F32R, tag="g0")
            nc.scalar.activation(
                out=g0[:], in_=sqd[:],
                func=mybir.ActivationFunctionType.Sqrt,
                bias=cj_sb[:, jt:jt + 1], scale=-2.0,
            )

            # software pipeline: the Y0T matmuls AND the rowmax update run one
            # step behind the gram. For the DVE this keeps tensor_sub (which
            # releases the psg PSUM slot) from queueing behind tensor_max
            # (which waits on the ACT sqrt) — otherwise the psg recycle stalls
            # the PE every other iteration.
            if jt > 0:
                pg0, pxt = prev
                if jt == 1:
                    nc.vector.tensor_copy(out=macc[:], in_=pg0[:].bitcast(F32))
                else:
                    nc.vector.tensor_max(macc[:], macc[:], pg0[:].bitcast(F32))
                for c in range(KT):
                    nc.tensor.matmul(
                        psy[c][:], pxt[:, bass.ts(c, 128)], pg0[:],
                        start=(jt == 1), stop=False,
                    )
            prev = (g0, xt)

        pg0, pxt = prev
        nc.vector.tensor_max(macc[:], macc[:], pg0[:].bitcast(F32))
        for c in range(KT):
            nc.tensor.matmul(
                psy[c][:], pxt[:, bass.ts(c, 128)], pg0[:],
                start=False, stop=True,
            )

        # tail, part 1: rowmax -> -1/rowmax per i sub-tile
        ninvs = []
        for s in range(NSUB):
            pst = ps_tr.tile([128, 128], F32, tag="tr")
            nc.tensor.transpose(pst[:], macc[:, bass.ts(s, 128)], ident[:])
            rm = sm_pool.tile([128, 1], F32, tag="rm")
            nc.vector.tensor_reduce(
                out=rm[:], in_=pst[:], axis=mybir.AxisListType.X,
                op=mybir.AluOpType.max,
            )
            nrm = sm_pool.tile([128, 1], F32, tag="nrm")
            nc.vector.tensor_scalar_mul(nrm[:], rm[:], -1.0)
            ninv = sm_pool.tile([128, 1], F32, tag="ninv", name=f"ninv{s}")
            nc.vector.reciprocal(ninv[:], nrm[:])  # -1/rowmax
            ninvs.append(ninv)

        # tail, part 2: Y0T out of PSUM (frees the psy banks for the next
        # i-block); the GEMM/scale/store closures are deferred into the next
        # i-block's instruction stream (emitted immediately on the last block)
        y0ts = []
        for c in range(KT):
            y0t = y0t_pool.tile([128, IB], F32R, tag="y0t", name=f"y0t{ib}_{c}")
            if c % 2 == 0:
                nc.vector.tensor_copy(out=y0t[:], in_=psy[c][:])
            else:
                nc.scalar.copy(out=y0t[:], in_=psy[c][:])
            y0ts.append(y0t)

        deferred = [
            (lambda ib=ib, s=s, y0ts=y0ts, ninv=ninvs[s]: emit_out(ib, s, y0ts, ninv))
            for s in range(NSUB)
        ]
        if ib == NIB - 1:
            for fn in deferred:
                fn()


_NC_CACHE = {}


def _build_nc():
    if "nc" in _NC_CACHE:
        return _NC_CACHE["nc"]
    nc = bacc.Bacc("TRN2", target_bir_lowering=False, debug=False, num_devices=M)
    x_d = nc.dram_tensor("x", [N, D], F32, kind="ExternalInput").ap()
    s_d = nc.dram_tensor("simT", [D, N], BF16, kind="ExternalInput").ap()
    cj_d = nc.dram_tensor("cj", [128, NJT], F32, kind="ExternalInput").ap()
    rib_d = nc.dram_tensor("riB", [128, R], F32, kind="ExternalInput").ap()
    w_d = nc.dram_tensor("w", [D, NOUT], F32, kind="ExternalInput").ap()
    w2b_d = nc.dram_tensor("w2B", [128, NOUT], F32, kind="ExternalInput").ap()
    out_d = nc.dram_tensor("out", [R, NOUT], F32, kind="ExternalOutput").ap()
    with tile.TileContext(nc) as tc, ExitStack() as ctx:
        build_kernel(ctx, tc, out_d, x_d, s_d, cj_d, rib_d, w_d, w2b_d)
    nc.compile()
    _NC_CACHE["nc"] = nc
    return nc


def make_in_maps(x, sim_feat, weight):
    x = np.ascontiguousarray(x, dtype=np.float32)
    w = np.ascontiguousarray(weight, dtype=np.float32)
    # the gram matmuls run in bf16: round sim on the host and derive ri/cj
    # from the ROUNDED values so the device computes exact distances of the
    # rounded vectors (error = distance perturbation only, ~1e-3 relative)
    sim_bf = np.asarray(sim_feat, dtype=np.float32).astype(ml_dtypes.bfloat16)

    sim64 = sim_bf.astype(np.float64)
    sq = (sim64 * sim64).sum(1)
    ss = sim64.sum(1)
    cj_full = (sq - 2.0 * EPS * ss + CLAMP).astype(np.float32)         # [N]
    ri_full = sq + 2.0 * EPS * ss + D * EPS * EPS                      # [N] f64
    colsum = x.astype(np.float64).sum(0)
    w2 = (colsum @ w.astype(np.float64)).astype(np.float32)
    w2B = np.ascontiguousarray(np.broadcast_to(w2, (128, NOUT)))

    in_maps = []
    for c in range(M):
        shift = c * R
        sim_c = np.ascontiguousarray(np.roll(sim_bf, -shift, axis=0).T)
        x_c = np.roll(x, -shift, axis=0)
        cj_c = np.ascontiguousarray(
            np.roll(cj_full, -shift).reshape(NJT, 128).T
        )                                                               # [128, NJT]
        ri_c = (ri_full[shift:shift + R] / 2.0).astype(np.float32)
        rib_c = np.ascontiguousarray(np.broadcast_to(ri_c, (128, R)))
        in_maps.append(
            {"x": x_c, "simT": sim_c, "cj": cj_c, "riB": rib_c,
             "w": w, "w2B": w2B}
        )
    return in_maps


def kernel(x, sim_feat, weight, _trace=False, **kw):
    nc = _build_nc()
    in_maps = make_in_maps(x, sim_feat, weight)
    res = run_bass_kernel_spmd(nc, in_maps, list(range(M)), trace=_trace, **kw)
    out = np.concatenate([res.results[c]["out"] for c in range(M)], axis=0)
    if _trace:
        return out, res
    return out


# revision 24
# speedup vs baseline: 1.4176x; 1.0118x over previous
"""Bass/Trainium2 kernel for nn_Graph_Layer (gnn_message_passing).

Reference math (N=8192, D=512):
    G0[i,j] = ||s_i - s_j + eps||_2   (pairwise distances, Gram trick)
    G = 1 - G0 / rowmax(G0)
    out = (G @ x) @ W

Decomposition (row-shard over 8 cores, 1024 rows each):
    sqd[i,j] = ri[i] + cj[j] - 2*gram[i,j]        (ri, cj host-precomputed)
    G0 = sqrt(sqd + CLAMP)                         (CLAMP covers tf32 noise on diag)
    rowmax[i] = max_j G0[i,j]
    (G @ x)[i,:] = colsum_x - Y0[i,:]/rowmax[i],   Y0 = G0 @ x
    out[i,:]  = w2 - (Y0 @ W)[i,:]/rowmax[i],      w2 = colsum_x @ W (host)

On device the distance strip is computed TRANSPOSED (sqd^T[j,i]) so G0 tiles
come out with j (the contraction dim of Y0) on partitions. The per-i "ri" term
is added by the Vector engine from a host-precomputed broadcast tile (riB)
instead of a 1-row PE matmul: a 1-row matmul costs the same PE cycles as a
full one (cost = output free size), and its LDWEIGHTS stalled the PE pipeline
every iteration, dropping the p-state.

Y0 is accumulated transposed (Y0T[c,i], stationary = x c-slices, moving = G0)
so no transposes are needed before the W GEMM; the GEMM output lands [i, n]
with i on partitions, where the -1/rowmax scale is a per-partition ACT scale
and w2 is a DVE add of a host broadcast tile.

Each core sees its own np.roll'ed copy of the inputs so local rows are always
[0,1024): a single uniform SPMD program runs on all 8 cores. All matmuls use
float32r (TF32 mode).
"""

import numpy as np
import ml_dtypes
from contextlib import ExitStack

import concourse.bass as bass
from concourse import bacc
import concourse.tile as tile
from concourse import mybir
from concourse.bass_utils import run_bass_kernel_spmd
from concourse.masks import make_identity

N, D, NOUT = 8192, 512, 512
M = 8                 # cores
R = N // M            # 1024 local rows per core
EPS = 1e-6
CLAMP = 0.3           # covers tf32 rounding noise on the diagonal; ~1e-4 rel effect off-diag
F32 = mybir.dt.float32
F32R = mybir.dt.float32r
BF16 = mybir.dt.bfloat16

KT = D // 128         # 4 contraction sub-tiles
NJT = N // 128        # 64 j tiles
IB = 512              # i block (free dim of the gram matmuls)
NIB = R // IB         # 2
NSUB = IB // 128      # 4 sub-tiles of 128 rows per i block

CH = 512              # S^T DMA chunk width (columns); chunk c covers j_tiles 4c..4c+3
NCH = N // CH


def build_kernel(ctx, tc, out_d, x_d, s_d, cj_d, rib_d, w_d, w2b_d):
    nc = tc.nc

    singles = ctx.enter_context(tc.tile_pool(name="singles", bufs=1))
    xt_pool = ctx.enter_context(tc.tile_pool(name="xt", bufs=4))
    g0_pool = ctx.enter_context(tc.tile_pool(name="g0", bufs=3))
    sqd_pool = ctx.enter_context(tc.tile_pool(name="sqd", bufs=3))
    y0t_pool = ctx.enter_context(tc.tile_pool(name="y0t", bufs=4))
    osb_pool = ctx.enter_context(tc.tile_pool(name="osb", bufs=4))
    sm_pool = ctx.enter_context(tc.tile_pool(name="sm", bufs=4))
    macc_pool = ctx.enter_context(tc.tile_pool(name="macc", bufs=2))
    ps_tr = ctx.enter_context(tc.tile_pool(name="ps_tr", bufs=1, space="PSUM"))
    ps_g = ctx.enter_context(tc.tile_pool(name="ps_g", bufs=2, space="PSUM"))
    ps_y = ctx.enter_context(tc.tile_pool(name="ps_y", bufs=1, space="PSUM"))
    ps_o = ctx.enter_context(tc.tile_pool(name="ps_o", bufs=1, space="PSUM"))

    # --- persistent SBUF tensors ---
    st = singles.tile([128, KT * N], BF16)            # S^T: [k*N + j] layout
    w_sb = singles.tile([128, KT * NOUT], F32R)       # W c-tiles
    cj_sb = singles.tile([128, NJT], F32)             # cj[t*128+p] at [p, t]
    rib_sb = singles.tile([128, R], F32)              # ri/2 broadcast across partitions
    w2b_sb = singles.tile([128, NOUT], F32)           # w2 broadcast across partitions
    ident = singles.tile([128, 128], F32)

    def load_st_chunk(c):
        for k in range(KT):
            nc.sync.dma_start(
                out=st[:, k * N + c * CH: k * N + (c + 1) * CH],
                in_=s_d[bass.ts(k, 128), c * CH:(c + 1) * CH],
            )

    # chunk 0 first: the first gram matmuls are gated on it; riB isn't needed
    # until the first DVE sub, cj until the first ACT, w2B until the tail
    load_st_chunk(0)
    nc.sync.dma_start(out=rib_sb[:], in_=rib_d)
    nc.sync.dma_start(out=cj_sb[:], in_=cj_d)
    nc.sync.dma_start(out=w2b_sb[:], in_=w2b_d)
    make_identity(nc, ident[:])  # only needed at the i-block tails

    def emit_out(ib, s, y0ts, ninv):
        """GEMM + scale + w2 + store for one 128-row output sub-tile."""
        pso = ps_o.tile([128, NOUT], F32, tag="o", name=f"pso{ib}_{s}")
        for c in range(KT):
            nc.tensor.matmul(
                pso[:],
                y0ts[c][:, bass.ts(s, 128)],
                w_sb[:, c * NOUT:(c + 1) * NOUT],
                start=(c == 0),
                stop=(c == KT - 1),
            )
        acted = osb_pool.tile([128, NOUT], F32, tag="osb", name=f"acted{ib}_{s}")
        nc.scalar.activation(
            out=acted[:], in_=pso[:],
            func=mybir.ActivationFunctionType.Copy, scale=ninv[:],
        )
        osb = osb_pool.tile([128, NOUT], F32, tag="osb", name=f"osb{ib}_{s}")
        nc.vector.tensor_add(osb[:], acted[:], w2b_sb[:])
        nc.sync.dma_start(out=out_d[bass.ts(ib * NSUB + s, 128), :], in_=osb[:])

    # --- main: per i-block: gram strip -> G0 -> Y0T accum -> normalize -> GEMM ---
    deferred = []  # output-side tail closures from the previous i-block
    for ib in range(NIB):
        icol0 = ib * IB  # local column offset into S^T / riB
        psy = [ps_y.tile([128, IB], F32, tag=f"y{c}", name=f"psy{c}")
               for c in range(KT)]
        macc = macc_pool.tile([128, IB], F32, tag="macc")
        hist = []

        for jt in range(NJT):
            # interleave the previous i-block's output tail into this block's
            # stream so its dependency latency hides under main-loop compute
            if deferred and jt >= 2 and jt % 2 == 0:
                deferred.pop(0)()
            xt = xt_pool.tile([128, D], F32R, tag="xt")
            nc.sync.dma_start(out=xt[:], in_=x_d[bass.ts(jt, 128), :].bitcast(F32R))

            if ib == 0:
                if jt == 0:
                    load_st_chunk(1)
                    load_st_chunk(2)
                elif jt % 4 == 0 and jt // 4 + 2 < NCH:
                    load_st_chunk(jt // 4 + 2)
                if jt == 32:
                    for kt in range(KT):
                        nc.sync.dma_start(
                            out=w_sb[:, kt * NOUT:(kt + 1) * NOUT],
                            in_=w_d[bass.ts(kt, 128), :].bitcast(F32R),
                        )

            psg = ps_g.tile([128, IB], F32, tag="g")
            for k in range(KT):
                nc.tensor.matmul(
                    psg[:],
                    st[:, k * N + jt * 128: k * N + jt * 128 + 128],
                    st[:, k * N + icol0: k * N + icol0 + IB],
                    start=(k == 0),
                    stop=(k == KT - 1),
                )
            # sqd = gram - ri/2  (broadcast tile; per-i term of the expansion);
            # PSUM -> SBUF, freeing the psg bank for the next gram group
            sqd = sqd_pool.tile([128, IB], F32, tag="sqd")
            nc.vector.tensor_sub(sqd[:], psg[:], rib_sb[:, icol0:icol0 + IB])

            # G0^T tile = sqrt(-2*sqd + cj[j])   (cj includes +CLAMP)
            g0 = g0_pool.tile([128, IB], F32R, tag="g0")
            nc.scalar.activation(
                out=g0[:], in_=sqd[:],
                func=mybir.ActivationFunctionType.Sqrt,
                bias=cj_sb[:, jt:jt + 1], scale=-2.0,
            )

            # software pipeline: the Y0T matmuls AND the rowmax update run TWO
            # steps behind the gram, giving the gram->sub->sqrt chain a full
            # extra iteration of slack before Y consumes g0. Emitting sub
            # before max also keeps the psg PSUM recycle off the ACT chain.
            if jt >= 2:
                pg0, pxt = hist[jt - 2]
                if jt == 2:
                    nc.vector.tensor_copy(out=macc[:], in_=pg0[:].bitcast(F32))
                else:
                    nc.vector.tensor_max(macc[:], macc[:], pg0[:].bitcast(F32))
                for c in range(KT):
                    nc.tensor.matmul(
                        psy[c][:], pxt[:, bass.ts(c, 128)], pg0[:],
                        start=(jt == 2), stop=False,
                    )
            hist.append((g0, xt))

        for tail_jt in (NJT - 2, NJT - 1):
            pg0, pxt = hist[tail_jt]
            nc.vector.tensor_max(macc[:], macc[:], pg0[:].bitcast(F32))
            for c in range(KT):
                nc.tensor.matmul(
                    psy[c][:], pxt[:, bass.ts(c, 128)], pg0[:],
                    start=False, stop=(tail_jt == NJT - 1),
                )

        # tail, part 1: rowmax -> -1/rowmax per i sub-tile
        ninvs = []
        for s in range(NSUB):
            pst = ps_tr.tile([128, 128], F32, tag="tr")
            nc.tensor.transpose(pst[:], macc[:, bass.ts(s, 128)], ident[:])
            rm = sm_pool.tile([128, 1], F32, tag="rm")
            nc.vector.tensor_reduce(
                out=rm[:], in_=pst[:], axis=mybir.AxisListType.X,
                op=mybir.AluOpType.max,
            )
            nrm = sm_pool.tile([128, 1], F32, tag="nrm")
            nc.vector.tensor_scalar_mul(nrm[:], rm[:], -1.0)
            ninv = sm_pool.tile([128, 1], F32, tag="ninv", name=f"ninv{s}")
            nc.vector.reciprocal(ninv[:], nrm[:])  # -1/rowmax
            ninvs.append(ninv)

        # tail, part 2: Y0T out of PSUM (frees the psy banks for the next
        # i-block); the GEMM/scale/store closures are deferred into the next
        # i-block's instruction stream (emitted immediately on the last block)
        y0ts = []
        for c in range(KT):
            y0t = y0t_pool.tile([128, IB], F32R, tag="y0t", name=f"y0t{ib}_{c}")
            if c % 2 == 0:
                nc.vector.tensor_copy(out=y0t[:], in_=psy[c][:])
            else:
                nc.scalar.copy(out=y0t[:], in_=psy[c][:])
            y0ts.append(y0t)

        deferred = [
            (lambda ib=ib, s=s, y0ts=y0ts, ninv=ninvs[s]: emit_out(ib, s, y0ts, ninv))
            for s in range(NSUB)
        ]
        if ib == NIB - 1:
            for fn in deferred:
                fn()


_NC_CACHE = {}


def _build_nc():
    if "nc" in _NC_CACHE:
        return _NC_CACHE["nc"]
    nc = bacc.Bacc("TRN2", target_bir_lowering=False, debug=False, num_devices=M)
    x_d = nc.dram_tensor("x", [N, D], F32, kind="ExternalInput").ap()
    s_d = nc.dram_tensor("simT", [D, N], BF16, kind="ExternalInput").ap()
    cj_d = nc.dram_tensor("cj", [128, NJT], F32, kind="ExternalInput").ap()
    rib_d = nc.dram_tensor("riB", [128, R], F32, kind="ExternalInput").ap()
    w_d = nc.dram_tensor("w", [D, NOUT], F32, kind="ExternalInput").ap()
    w2b_d = nc.dram_tensor("w2B", [128, NOUT], F32, kind="ExternalInput").ap()
    out_d = nc.dram_tensor("out", [R, NOUT], F32, kind="ExternalOutput").ap()
    with tile.TileContext(nc) as tc, ExitStack() as ctx:
        build_kernel(ctx, tc, out_d, x_d, s_d, cj_d, rib_d, w_d, w2b_d)
    nc.compile()
    _NC_CACHE["nc"] = nc
    return nc


def make_in_maps(x, sim_feat, weight):
    x = np.ascontiguousarray(x, dtype=np.float32)
    w = np.ascontiguousarray(weight, dtype=np.float32)
    # the gram matmuls run in bf16: round sim on the host and derive ri/cj
    # from the ROUNDED values so the device computes exact distances of the
    # rounded vectors (error = distance perturbation only, ~1e-3 relative)
    sim_bf = np.asarray(sim_feat, dtype=np.float32).astype(ml_dtypes.bfloat16)

    sim64 = sim_bf.astype(np.float64)
    sq = (sim64 * sim64).sum(1)
    ss = sim64.sum(1)
    cj_full = (sq - 2.0 * EPS * ss + CLAMP).astype(np.float32)         # [N]
    ri_full = sq + 2.0 * EPS * ss + D * EPS * EPS                      # [N] f64
    colsum = x.astype(np.float64).sum(0)
    w2 = (colsum @ w.astype(np.float64)).astype(np.float32)
    w2B = np.ascontiguousarray(np.broadcast_to(w2, (128, NOUT)))

    in_maps = []
    for c in range(M):
        shift = c * R
        sim_c = np.ascontiguousarray(np.roll(sim_bf, -shift, axis=0).T)
        x_c = np.roll(x, -shift, axis=0)
        cj_c = np.ascontiguousarray(
            np.roll(cj_full, -shift).reshape(NJT, 128).T
        )                                                               # [128, NJT]
        ri_c = (ri_full[shift:shift + R] / 2.0).astype(np.float32)
        rib_c = np.ascontiguousarray(np.broadcast_to(ri_c, (128, R)))
        in_maps.append(
            {"x": x_c, "simT": sim_c, "cj": cj_c, "riB": rib_c,
             "w": w, "w2B": w2B}
        )
    return in_maps


def kernel(x, sim_feat, weight, _trace=False, **kw):
    nc = _build_nc()
    in_maps = make_in_maps(x, sim_feat, weight)
    res = run_bass_kernel_spmd(nc, in_maps, list(range(M)), trace=_trace, **kw)
    out = np.concatenate([res.results[c]["out"] for c in range(M)], axis=0)
    if _trace:
        return out, res
    return out
